# revision 16
# baseline (speedup 1.0000x reference)
"""Trainium2 Bass kernel for SAGAN-style spatial self-attention (B=4, C=256,
H=W=64): y = gamma * attention(x) + x with 1x1-conv q/k/v projections.

Sharding: 8 cores = 4 batch elements x 2 query-row halves; each core computes
its [2048, 256] slice of y^T independently (no collectives needed).

Per-core algorithm, shaped around TensorE's out = lhsT.T @ rhs form:
  - The host passes xf column-PERMUTED as [query-block | complement]
    (attention is order-invariant over keys j as long as k and v use the same
    order), so q is just the projection of xf's first half and no separate
    query input is needed. Host prep also pre-transposes/stacks the weights:
    wqk = [Wq^T | Wk^T], wvt = gamma * Wv^T, and prefolds gamma*bv + x^T.
  - q, k produced directly in [d, n] layout; the first half of k comes out of
    the same stacked [e, 64] matmuls that produce q.
  - v^T produced directly as xf.T @ (gamma*Wv^T) -- no device transpose;
    an extra ones-column on v^T makes the out accumulation also produce the
    softmax denominator s_i for free.
  - energy is computed TRANSPOSED, e^T[j, i] = (k-chunk).T @ q, so exp reads
    it with j on partitions; no max-subtraction is needed (|e| <= ~40, exp
    accumulates in fp32 psum).
  - The K=32 energy contraction would waste 3/4 of the PE array, so the four
    j-chunks of each group run CONCURRENTLY via row tiling
    (tile_position=(32g, 0)), with k and q restriped/replicated into the four
    32-partition groups by SBUF->SBUF DMA:
      k4[32g + d, jj, c] = k[d, (4*jj + g)*128 + c];  q4[32g + d, i] = q[d, i]
    Each group's four matmuls land in four separate PSUM banks and one
    batched [128, 4, 512] exp evacuates them in a single ScalarE instruction.
  - out^T[i, c] = sum_j S[j, i] (gamma*v^T)[j, c] accumulates over all 32
    j-chunks in psum; column 256 holds s_i; epilogue does
    y^T = out^T * (1/s_i) + (x^T + gamma*bv).
All matmuls run in bf16 (1 cycle/row on the PE; float32r measured 2 cycles/row
on hardware). PSUM accumulation, softmax and the epilogue stay fp32.
The attention loop is software-pipelined with a one-group skew so TensorE
streams continuously while ScalarE exps the previous group.
"""

import sys

import numpy as np

for _p in ("/opt/trn_rl_repo", "/root/.axon_site", "/root/.axon_site/_ro/pypackages"):
    if _p not in sys.path:
        sys.path.insert(0, _p)

B, C, HW, N, D = 4, 256, 64, 4096, 32
NQ = N // 2
IBLK = 512
NIB = NQ // IBLK     # 4 i-blocks per core
NJ = N // 128        # 32 j-chunks
JG = 4               # j-chunks per group == row-tiling factor
NIC = IBLK // 128    # 4 i-chunks of 128 per i-block

_NC_CACHE = {}


def _build_nc():
    import concourse.bass as bass
    import concourse.tile as tile
    from concourse import bacc, mybir

    f32 = mybir.dt.float32
    bf16 = mybir.dt.bfloat16
    Exp = mybir.ActivationFunctionType.Exp
    ts = bass.ts

    nc = bacc.Bacc(None, target_bir_lowering=False, debug=False)

    xf_d = nc.declare_dram_parameter("xf", [C, N], bf16, isOutput=False)
    xbt_d = nc.declare_dram_parameter("xbt", [NQ, C], f32, isOutput=False)
    wqk_d = nc.declare_dram_parameter("wqk", [C, 2 * D], bf16, isOutput=False)
    wkt_d = nc.declare_dram_parameter("wkt", [C, D], bf16, isOutput=False)
    wvt_d = nc.declare_dram_parameter("wvt", [C, C], bf16, isOutput=False)
    bqk_d = nc.declare_dram_parameter("bqk", [2 * D, 1], f32, isOutput=False)
    bk_d = nc.declare_dram_parameter("bk", [D, 1], f32, isOutput=False)
    out_d = nc.declare_dram_parameter("out", [NQ, C], f32, isOutput=True)

    XCH = 4

    with tile.TileContext(nc) as tc:
        with (
            tc.tile_pool(name="const", bufs=1) as cpool,
            tc.tile_pool(name="sblk", bufs=4) as spool,
            tc.tile_pool(name="ytile", bufs=3) as ypool,
            tc.tile_pool(name="small", bufs=4) as rpool,
        ):
            prime_in = rpool.tile([1, 2], f32, tag="prime", name="prime_in")
            prime_out = rpool.tile([1, 2], f32, tag="prime", name="prime_out")
            nc.vector.memset(prime_in[:], 0.0)
            nc.scalar.activation(prime_out[:], prime_in[:], Exp)

            xf_sb = cpool.tile([128, 2, N], bf16)
            xbt_sb = cpool.tile([128, NQ // 128, C], f32)
            wqk_sb = cpool.tile([128, 2, 2 * D], bf16)
            wk_sb = cpool.tile([128, 2, D], bf16)
            wv_sb = cpool.tile([128, 2, C], bf16)
            bqk_sb = cpool.tile([2 * D, 1], f32)
            bk_sb = cpool.tile([D, 1], f32)
            XW = N // XCH
            for ec in range(2):
                nc.gpsimd.dma_start(wqk_sb[:, ec, :], wqk_d[ts(ec, 128), :])
            for ec in range(2):
                nc.gpsimd.dma_start(
                    xf_sb[:, ec, ts(0, XW)], xf_d[ts(ec, 128), ts(0, XW)]
                )
            nc.gpsimd.dma_start(bqk_sb[:], bqk_d[:])
            for ec in range(2):
                nc.gpsimd.dma_start(wk_sb[:, ec, :], wkt_d[ts(ec, 128), :])
            for ec in range(2):
                nc.gpsimd.dma_start(
                    xf_sb[:, ec, ts(1, XW)], xf_d[ts(ec, 128), ts(1, XW)]
                )
            nc.gpsimd.dma_start(bk_sb[:], bk_d[:])
            for ec in range(2):
                nc.gpsimd.dma_start(wv_sb[:, ec, :], wvt_d[ts(ec, 128), :])
            for cc in range(2, XCH):
                for ec in range(2):
                    nc.gpsimd.dma_start(
                        xf_sb[:, ec, ts(cc, XW)],
                        xf_d[ts(ec, 128), ts(cc, XW)],
                    )

            # ---- projections -------------------------------------------------
            k_sb = cpool.tile([D, N], bf16)
            q_sb = cpool.tile([D, NQ], bf16)
            k4_sb = cpool.tile([128, NJ // 4, 128], bf16)
            q4_sb = cpool.tile([128, NQ], bf16)
            vt_sb = cpool.tile([128, NJ, C + 2], bf16)
            ones_sb = cpool.tile([128, NJ, 2], f32)
            nc.vector.memset(ones_sb[:], 1.0)
            nc.vector.tensor_copy(vt_sb[:, :, C : C + 2], ones_sb[:])

            with tc.tile_pool(name="psA", bufs=3, space="PSUM") as psA:
                for qc in range(NQ // 512):
                    ps = psA.tile([2 * D, 512], f32, tag="psA", name=f"psq{qc}")
                    for ec in range(2):
                        nc.tensor.matmul(
                            ps[:],
                            wqk_sb[:, ec, :],
                            xf_sb[:, ec, ts(qc, 512)],
                            start=(ec == 0),
                            stop=(ec == 1),
                        )
                    nc.vector.tensor_scalar_add(
                        q_sb[:, ts(qc, 512)], ps[0:D, :], bqk_sb[0:D, :]
                    )
                    nc.vector.tensor_scalar_add(
                        k_sb[:, ts(qc, 512)], ps[D : 2 * D, :], bqk_sb[D : 2 * D, :]
                    )
                for jc in range(NQ // 512, N // 512):
                    ps = psA.tile([D, 512], f32, tag="psA", name=f"psk{jc}")
                    for ec in range(2):
                        nc.tensor.matmul(
                            ps[:],
                            wk_sb[:, ec, :],
                            xf_sb[:, ec, ts(jc, 512)],
                            start=(ec == 0),
                            stop=(ec == 1),
                        )
                    nc.vector.tensor_scalar_add(k_sb[:, ts(jc, 512)], ps[:], bk_sb[:])
                # restripe k/q into the four 32-partition row-tile groups
                k_r = k_sb[:, :].rearrange("d (jj f c) -> d f jj c", f=4, c=128)
                for g in range(4):
                    nc.gpsimd.dma_start(
                        k4_sb[32 * g : 32 * (g + 1), :, :], k_r[:, g, :, :]
                    )
                    nc.gpsimd.dma_start(q4_sb[32 * g : 32 * (g + 1), :], q_sb[:, :])
                for j in range(NJ):
                    ps = psA.tile([128, C], f32, tag="psA", name=f"psv{j}")
                    for ec in range(2):
                        nc.tensor.matmul(
                            ps[:],
                            xf_sb[:, ec, ts(j, 128)],
                            wv_sb[:, ec, :],
                            start=(ec == 0),
                            stop=(ec == 1),
                        )
                    nc.vector.tensor_copy(vt_sb[:, j, 0:C], ps[:])

            nc.gpsimd.dma_start(
                xbt_sb[:], xbt_d[:].rearrange("(t p) c -> p t c", p=128)
            )

            # ---- attention (pipelined, row-tiled energy) ---------------------
            groups = [(ib, jg) for ib in range(NIB) for jg in range(NJ // JG)]
            with (
                tc.tile_pool(name="psE", bufs=1, space="PSUM") as psE,
                tc.tile_pool(name="psO", bufs=NIC, space="PSUM") as psO,
            ):
                opss = {}
                s_tiles = {}

                def emit_energy_exp(t):
                    ib, jg = groups[t]
                    if jg == 0:
                        opss[ib] = [
                            psO.tile([128, C + 2], f32, tag="psO", name=f"ops{ib}_{i2}")
                            for i2 in range(NIC)
                        ]
                    eps = psE.tile([128, JG, IBLK], f32, tag="psE", name=f"eps{t}")
                    for g in range(JG):
                        nc.tensor.matmul(
                            eps[:, g, :],
                            k4_sb[32 * g : 32 * (g + 1), jg, :],
                            q4_sb[32 * g : 32 * (g + 1), ts(ib, IBLK)],
                            start=True,
                            stop=True,
                            tile_position=(32 * g, 0),
                        )
                    s_t = spool.tile([128, JG, IBLK], bf16, tag="sblk", name=f"s{t}")
                    nc.scalar.activation(s_t[:], eps[:], Exp)
                    s_tiles[t] = s_t

                def emit_out(t):
                    ib, jg = groups[t]
                    s_t = s_tiles.pop(t)
                    for ic2 in range(NIC):
                        for g in range(JG):
                            j = jg * JG + g
                            nc.tensor.matmul(
                                opss[ib][ic2][:],
                                s_t[:, g, ts(ic2, 128)],
                                vt_sb[:, j, :],
                                start=(j == 0),
                                stop=(j == NJ - 1),
                            )
                    if jg == NJ // JG - 1:
                        for ic2 in range(NIC):
                            ic = ib * NIC + ic2
                            ops = opss[ib][ic2]
                            r = rpool.tile(
                                [128, 1], f32, tag="small", name=f"r{ib}_{ic2}"
                            )
                            nc.vector.reciprocal(r[:], ops[:, C : C + 1])
                            y = ypool.tile([128, C], f32, tag="ytile", name=f"y{ic}")
                            nc.vector.tensor_scalar_mul(y[:], ops[:, 0:C], r[:])
                            nc.vector.tensor_add(y[:], y[:], xbt_sb[:, ic, :])
                            nc.gpsimd.dma_start(out_d[ts(ic, 128), :], y[:])

                emit_energy_exp(0)
                for t in range(1, len(groups)):
                    emit_energy_exp(t)
                    emit_out(t - 1)
                emit_out(len(groups) - 1)

    nc.compile()
    return nc


def _get_nc():
    if "nc" not in _NC_CACHE:
        _NC_CACHE["nc"] = _build_nc()
    return _NC_CACHE["nc"]


def kernel(x, Wq, bq, Wk, bk, Wv, bv, gamma):
    import ml_dtypes
    from concourse.bass_utils import run_bass_kernel_spmd

    bf = ml_dtypes.bfloat16
    x = np.asarray(x, dtype=np.float32)
    gamma_v = float(np.asarray(gamma).reshape(-1)[0])
    xf = x.reshape(B, C, N)
    wqk = np.ascontiguousarray(
        np.concatenate([np.asarray(Wq, np.float32).T, np.asarray(Wk, np.float32).T], axis=1).astype(bf)
    )
    wkt = np.ascontiguousarray(np.asarray(Wk, np.float32).T.astype(bf))
    wvt = np.ascontiguousarray((gamma_v * np.asarray(Wv, np.float32).T).astype(bf))
    bqk_c = np.concatenate(
        [np.asarray(bq, np.float32).reshape(D, 1), np.asarray(bk, np.float32).reshape(D, 1)]
    )
    bk_c = np.asarray(bk, np.float32).reshape(D, 1).copy()
    gbv = (gamma_v * np.asarray(bv, np.float32))[None, :]

    in_maps = []
    for core in range(8):
        b, h = divmod(core, 2)
        sl = slice(h * NQ, (h + 1) * NQ)
        xb = xf[b]
        xperm = np.ascontiguousarray(
            np.concatenate([xb[:, sl], xb[:, 0 : h * NQ], xb[:, (h + 1) * NQ :]], axis=1)
        )
        xb16 = xperm.astype(bf)
        in_maps.append(
            {
                "xf": xb16,
                "xbt": np.ascontiguousarray(xb[:, sl].T + gbv),
                "wqk": wqk,
                "wkt": wkt,
                "wvt": wvt,
                "bqk": bqk_c,
                "bk": bk_c,
            }
        )

    nc = _get_nc()
    res = run_bass_kernel_spmd(nc, in_maps, core_ids=list(range(8)))
    y = np.empty((B, C, N), np.float32)
    for core in range(8):
        b, h = divmod(core, 2)
        y[b][:, h * NQ : (h + 1) * NQ] = res.results[core]["out"].T
    return y.reshape(B, C, HW, HW)
